# revision 1
# baseline (speedup 1.0000x reference)
"""BalancedWeightClusterLoss on 8 Trainium2 NeuronCores (Bass/Tile).

Reference computation (per channel c of weight [C, K], scale [C]):
    mean, std(ddof=1) over K
    lower = mean - 2*std ; step = 4*std/15
    idx = clip((w - lower)/step, 0, 14) -> int (trunc == floor, values >= 0)
    target = scale * (idx - 7)
    loss = sum |w - target|

Kernel identities (per-channel; r = 1/step, q = s*r, jc7 = idx-7):
    z    = w*r - nb1,  nb1 = mean*r - 7      (= (w-lower)/step - 0.5;
                                              the -0.5 makes round == floor)
    jc7  = clip(round(z) - 7, -7, 7)         (round = +2^23 - (2^23+7) in one
                                              dual-op ts, f32 internal)
    x    = q*jc7 - w*r   =>  |w - target| = step*|x|
    sum|x| = 2*sum(max(q*jc7, w*r)) - sum(w*r) - q*sum(jc7)
    => block loss = step*sum|x| = 2*sum(max(s*jc7, w)) - sum(w) - s*sum(jc7)
       (step*r = 1 and step*q = s pull the scaling out of every term)

Engine split (measured costs): ACT does the f32 reads (Sum(w), Sum(w^2),
and the affine z-pass via Identity with per-partition scale/bias APs);
DVE does round/clip (dual-op bf16 tensor_scalar), Sum(jc7) accum, and
one scalar_tensor_tensor computing max(s*jc7, w) with sum-accumulate.

Sharding: channels 4096 -> 512 per core (8 cores); per core 4 blocks of
128 partitions, w rows SBUF-resident in f32 (single HBM read). Host
sums the 8 x [128, 4] partial losses in float64.
"""
import numpy as np

import concourse.bacc as bacc
import concourse.tile as tile
from concourse import mybir
from concourse.bass_utils import run_bass_kernel_spmd

f32 = mybir.dt.float32
bf16 = mybir.dt.bfloat16
Alu = mybir.AluOpType
Act = mybir.ActivationFunctionType

# problem shape (hardcoded per contest contract)
CFULL, K = 4096, 16384
NCORES = 8
CSH = CFULL // NCORES          # 512 channels per core
P = 128                        # SBUF partitions
NBLK = CSH // P                # 4 blocks per core
CH = 4096                      # chunk width (free dim)
NCH = K // CH                  # 4 chunks per block

RND = float(2 ** 23)           # f32 round-to-int bias
RND7 = float(2 ** 23 + 7)      # fold the -7 shift into the round
INV_N = 1.0 / K
# step = (4/15) * std_unbiased = sqrt(varr * K2), varr = E[w^2]-mean^2
K2 = (4.0 / 15.0) ** 2 * (K / (K - 1.0))

_PROGRAM = None


def _build(repeat=1):
    nc = bacc.Bacc("TRN2", target_bir_lowering=False, debug=False,
                   num_devices=NCORES)
    w_ext = nc.dram_tensor("w", [CSH, K], f32, kind="ExternalInput")
    s_ext = nc.dram_tensor("s", [CSH, 1], f32, kind="ExternalInput")
    out_ext = nc.dram_tensor("out", [P, NBLK], f32, kind="ExternalOutput")

    with tile.TileContext(nc) as tc:
        with (
            tc.tile_pool(name="wpool", bufs=2) as wpool,
            tc.tile_pool(name="chunks", bufs=2) as chunks,
            tc.tile_pool(name="scrp", bufs=2) as scrp,
            tc.tile_pool(name="stats", bufs=2) as stats,
            tc.tile_pool(name="minis", bufs=2) as minis,
            tc.tile_pool(name="outp", bufs=1) as outp,
        ):
            out_sb = outp.tile([P, NBLK], f32)

            for b in [bb % NBLK for bb in range(NBLK * repeat)]:
                rows = slice(b * P, (b + 1) * P)
                w = wpool.tile([P, K], f32)
                sblk = minis.tile([P, 1], f32)
                nc.sync.dma_start(sblk[:], s_ext[rows, :])

                su_acc = stats.tile([P, NCH], f32)
                sq_acc = stats.tile([P, NCH], f32)
                for ch in range(NCH):
                    sl = slice(ch * CH, (ch + 1) * CH)
                    nc.sync.dma_start(w[:, sl], w_ext[rows, sl])
                    # Sum(w), Sum(w^2) on ACT (f32 reads)
                    scr = scrp.tile([P, CH], bf16, tag="scr")
                    nc.scalar.activation(scr[:], w[:, sl], Act.Copy,
                                         accum_out=su_acc[:, ch:ch + 1])
                    scr2 = scrp.tile([P, CH], bf16, tag="scr")
                    nc.scalar.activation(scr2[:], w[:, sl], Act.Square,
                                         accum_out=sq_acc[:, ch:ch + 1])

                # per-channel stats -> scalars
                SU = minis.tile([P, 1], f32)
                nc.vector.tensor_reduce(SU[:], su_acc[:], mybir.AxisListType.X,
                                        Alu.add)
                SQ = minis.tile([P, 1], f32)
                nc.vector.tensor_reduce(SQ[:], sq_acc[:], mybir.AxisListType.X,
                                        Alu.add)
                mean = minis.tile([P, 1], f32)
                nc.vector.tensor_scalar(mean[:], SU[:], INV_N, None, Alu.mult)
                E2 = minis.tile([P, 1], f32)
                nc.vector.tensor_scalar(E2[:], SQ[:], INV_N, None, Alu.mult)
                nvar = minis.tile([P, 1], f32)
                # nvar = mean*mean - E2  (negated biased variance)
                nc.vector.scalar_tensor_tensor(nvar[:], mean[:], mean[:], E2[:],
                                               Alu.mult, Alu.subtract)
                step = minis.tile([P, 1], f32)
                nc.scalar.activation(step[:], nvar[:], Act.Sqrt,
                                     bias=0.0, scale=-K2)
                r = minis.tile([P, 1], f32)
                nc.vector.reciprocal(r[:], step[:])
                nnb1 = minis.tile([P, 1], f32)
                # nnb1 = 7 - mean*r  (Identity's bias ADDS, so carry negated)
                nc.vector.tensor_scalar(nnb1[:], mean[:], r[:], -1.0,
                                        Alu.mult, Alu.mult)
                nc.vector.tensor_scalar(nnb1[:], nnb1[:], 7.0, None, Alu.add)

                au_dummy = None
                ajc = stats.tile([P, NCH], f32)
                am = stats.tile([P, NCH], f32)
                for ch in range(NCH):
                    sl = slice(ch * CH, (ch + 1) * CH)
                    # z = w*r + (7 - mean*r) on ACT -> bf16
                    z = chunks.tile([P, CH], bf16, tag="z")
                    nc.scalar.activation(z[:], w[:, sl], Act.Identity,
                                         bias=nnb1[:], scale=r[:])
                    # z = round(z) - 7   (internal-f32 2^23 trick, in place)
                    nc.vector.tensor_scalar(z[:], z[:], RND, RND7,
                                            Alu.add, Alu.subtract)
                    # jc7 = clip(z, -7, 7), accum Sum -> needs single-op form:
                    # do clip as dual op (no accum), then Sum on the STT? No:
                    # clip dual + separate bypass-accum would cost a pass, so
                    # clip with max only + min folded into accum op0:
                    # jc7 = min(max(z, -7), 7) with accum Sum(jc7):
                    nc.vector.tensor_scalar(z[:], z[:], -7.0, None, Alu.max)
                    jc7b = chunks.tile([P, CH], bf16, tag="jc7b")
                    nc.vector.tensor_scalar(jc7b[:], z[:], 7.0,
                                            0.0, Alu.min, Alu.add,
                                            accum_out=ajc[:, ch:ch + 1])
                    # m = max(s*jc7, w); Sum(m)  (in place over jc7b)
                    nc.vector.scalar_tensor_tensor(jc7b[:], jc7b[:], sblk[:],
                                                   w[:, sl], Alu.mult, Alu.max,
                                                   accum_out=am[:, ch:ch + 1])

                AJC = minis.tile([P, 1], f32)
                nc.vector.tensor_reduce(AJC[:], ajc[:], mybir.AxisListType.X,
                                        Alu.add)
                AM = minis.tile([P, 1], f32)
                nc.vector.tensor_reduce(AM[:], am[:], mybir.AxisListType.X,
                                        Alu.add)
                t1 = minis.tile([P, 1], f32)
                # t1 = 2*AM - SU
                nc.vector.tensor_scalar(t1[:], AM[:], 2.0, SU[:],
                                        Alu.mult, Alu.subtract)
                t2 = minis.tile([P, 1], f32)
                nc.vector.tensor_scalar(t2[:], AJC[:], sblk[:], None, Alu.mult)
                # out = t1 - t2
                nc.vector.scalar_tensor_tensor(out_sb[:, b:b + 1], t2[:],
                                               -1.0, t1[:],
                                               Alu.mult, Alu.add)

            nc.sync.dma_start(out_ext[:], out_sb[:])

    nc.compile()
    return nc


def _get_program():
    global _PROGRAM
    if _PROGRAM is None:
        _PROGRAM = _build()
    return _PROGRAM


def kernel(weight, scale):
    w = np.ascontiguousarray(np.asarray(weight, dtype=np.float32))
    s = np.ascontiguousarray(np.asarray(scale, dtype=np.float32)).reshape(CFULL, 1)
    assert w.shape == (CFULL, K), w.shape

    nc = _get_program()
    in_maps = [
        {"w": w[i * CSH:(i + 1) * CSH], "s": s[i * CSH:(i + 1) * CSH]}
        for i in range(NCORES)
    ]
    res = run_bass_kernel_spmd(nc, in_maps, list(range(NCORES)))
    total = 0.0
    for i in range(NCORES):
        total += res.results[i]["out"].astype(np.float64).sum()
    return np.float32(total)



# revision 3
# speedup vs baseline: 1.6585x; 1.6585x over previous
"""BalancedWeightClusterLoss on 8 Trainium2 NeuronCores (Bass/Tile).

Reference computation (per channel c of weight [C, K], scale [C]):
    mean, std(ddof=1) over K
    lower = mean - 2*std ; step = 4*std/15
    idx = clip((w - lower)/step, 0, 14) -> int (trunc == floor here)
    target = scale * (idx - 7)
    loss = sum |w - target|

Kernel derivation (per channel; r = 1/step, b1 = 7 - mean*r):
    idx = floor((w-lower)*r) = round(w*r + b1)       (round(x-.5)==floor(x))
    jc7 = clip(round(z), 0, 14) - 7,  z = w*r + b1
    loss = sum |w - s*jc7|

Engine split:
    ACT:  Copy(w_f32 -> w_f16) + riding accumulator => Sum(w)
          Square(w_f32 -> scratch) + accumulator    => Sum(w^2)
          Rsqrt for r (tiny)
    DVE:  z = w_f16*r + b1      (tensor_scalar dual, f16, 4x mode)
          custom fused op VQ_LOSS_ANT (registered at import):
             j = min(max(z + 2^23, 2^23) - (2^23+7), 7)   [f32 internal round]
             out = |w - s*j|, accum_out = per-channel sum  (1 elem/cycle,
             and the loss reduction rides the same pass for free)
    All reductions ride engine accumulators; no separate reduce passes.

Sharding: channels 4096 -> 512 per core (8 cores) x 4 row-blocks of 128
partitions. w is read from HBM exactly once (memory roofline ~94us/core).
Host sums the 8 x [128, 4] partial losses in float64.
"""
import numpy as np

import concourse.bacc as bacc
import concourse.tile as tile
from concourse import mybir
from concourse.bass_utils import run_bass_kernel_spmd

f32 = mybir.dt.float32
f16 = mybir.dt.float16
bf16 = mybir.dt.bfloat16
Alu = mybir.AluOpType
Act = mybir.ActivationFunctionType

# problem shape (hardcoded per contest contract)
CFULL, K = 4096, 16384
NCORES = 8
CSH = CFULL // NCORES          # 512 channels per core
P = 128                        # SBUF partitions
NBLK = CSH // P                # 4 row-blocks per core
CH = 4096                      # phase-1 chunk (f32 DMA + ACT passes)
NCH = K // CH                  # 4
CH2 = 8192                     # phase-2 chunk (DVE passes)
NCH2 = K // CH2                # 2

RND = float(2 ** 23)           # f32 round-to-int bias
RND7 = float(2 ** 23 + 7)
INV_K = 1.0 / K
# step^2 = K2 * var_biased ; var_b = E[w^2] - mean^2
K2 = (4.0 / 15.0) ** 2 * (K / (K - 1.0))

_PROGRAM = None


def _vq_ref(in0, in1, c0, c1, c2):
    """numpy reference for the custom DVE op (CoreSim executes this)."""
    z32 = np.asarray(in0, np.float32)
    v = (z32 + np.float32(c0)).astype(np.float32)
    v2 = np.maximum(v, np.float32(c0))
    j0 = (v2 - np.float32(c2)).astype(np.float32)
    j = np.minimum(j0, np.float32(c2 - c0))
    t = (j * np.asarray(c1, np.float32)).astype(np.float32)
    ae = np.abs(np.asarray(in1, np.float32) - t)
    return ae, ae.sum(axis=1, keepdims=True)


def _register_vq_op():
    """Register the fused loss op in concourse's custom-DVE table (runtime
    append; the uop program is compiled into the per-NEFF DVE table)."""
    import concourse.dve_ops as D
    from concourse.dve_spec import (
        Spec, Src0, Src1, C0, C1, C2, maxx, minn, Bin, AluOp, lower,
        _has_src1,
    )
    from concourse.dve_uop import DveOpSpec
    name = "VQ_LOSS_ANT"
    if name in D._SUB_OPCODE_FOR_NAME:
        for op in D.OPS:
            if op.name == name:
                return op
    v = Src0 + C0              # round(z) + 2^23   (f32 internal)
    v2 = maxx(v, C0)           # clip low: round(z) >= 0
    j0 = v2 - C2               # max(round(z),0) - 7
    j = minn(j0, C2 - C0)      # min(..., 7)  (C2-C0 auto-hoisted const)
    t = j * C1                 # s * jc7
    ae = Bin(AluOp.ABSOLUTE_DIFF, Src1, t)   # |w - s*jc7|
    spec = Spec(body=ae, accum=AluOp.ADD, reference=_vq_ref)
    row = D._CUSTOM_DVE_ROW_BASE + len(D.OPS)
    assert row < 0x20, "custom DVE row overflow"
    shas = {}
    for ver in ("v3", "v4"):
        s = DveOpSpec(name=name, opcode=row,
                      uops=lower(spec, ver=ver), rd1_en=_has_src1(spec))
        shas[ver] = s.sha(ver)
    op = D.DveOp(name, spec, subdim=False, uops_sha=shas)
    D.OPS.append(op)
    D._SUB_OPCODE_FOR_NAME[name] = row
    D.CUSTOM_DVE_SPECS[name] = spec
    return op


def _build():
    vq = _register_vq_op()
    nc = bacc.Bacc("TRN2", target_bir_lowering=False, debug=False,
                   num_devices=NCORES)
    w_ext = nc.dram_tensor("w", [CSH, K], f32, kind="ExternalInput")
    s_ext = nc.dram_tensor("s", [CSH, 1], f32, kind="ExternalInput")
    out_ext = nc.dram_tensor("out", [P, NBLK], f32, kind="ExternalOutput")

    with tile.TileContext(nc) as tc:
        with (
            tc.tile_pool(name="w32p", bufs=3) as w32p,
            tc.tile_pool(name="w16p", bufs=2) as w16p,
            tc.tile_pool(name="zp", bufs=2) as zp,
            tc.tile_pool(name="scrp", bufs=2) as scrp,
            tc.tile_pool(name="minis", bufs=2) as minis,
            tc.tile_pool(name="outp", bufs=1) as outp,
        ):
            out_sb = outp.tile([P, NBLK], f32)

            for b in range(NBLK):
                rows = slice(b * P, (b + 1) * P)
                sblk = minis.tile([P, 1], f32)
                nc.sync.dma_start(sblk[:], s_ext[rows, :])

                w16 = w16p.tile([P, K], f16)
                su = minis.tile([P, NCH], f32)
                sq = minis.tile([P, NCH], f32)
                for c in range(NCH):
                    sl = slice(c * CH, (c + 1) * CH)
                    w32 = w32p.tile([P, CH], f32, tag="w32")
                    nc.sync.dma_start(w32[:], w_ext[rows, sl])
                    # conversion pass carries Sum(w)
                    nc.scalar.activation(w16[:, sl], w32[:], Act.Copy,
                                         accum_out=su[:, c:c + 1])
                    # square pass carries Sum(w^2); output is scratch
                    gb = scrp.tile([P, CH], bf16, tag="gb")
                    nc.scalar.activation(gb[:], w32[:], Act.Square,
                                         accum_out=sq[:, c:c + 1])

                # per-channel params: r = 1/step, b1 = 7 - mean*r
                SU = minis.tile([P, 1], f32)
                nc.vector.tensor_reduce(SU[:], su[:], mybir.AxisListType.X,
                                        Alu.add)
                SQ = minis.tile([P, 1], f32)
                nc.vector.tensor_reduce(SQ[:], sq[:], mybir.AxisListType.X,
                                        Alu.add)
                mean = minis.tile([P, 1], f32)
                nc.vector.tensor_scalar(mean[:], SU[:], INV_K, None, Alu.mult)
                E2 = minis.tile([P, 1], f32)
                nc.vector.tensor_scalar(E2[:], SQ[:], INV_K, None, Alu.mult)
                nvar = minis.tile([P, 1], f32)
                # nvar = mean*mean - E2  (= -var_biased)
                nc.vector.scalar_tensor_tensor(nvar[:], mean[:], mean[:],
                                               E2[:], Alu.mult, Alu.subtract)
                step = minis.tile([P, 1], f32)
                # step = sqrt(K2*var_b) = Sqrt(-K2 * nvar)
                nc.scalar.activation(step[:], nvar[:], Act.Sqrt,
                                     bias=0.0, scale=-K2)
                r = minis.tile([P, 1], f32)
                nc.vector.reciprocal(r[:], step[:])
                b1 = minis.tile([P, 1], f32)
                # b1 = 7 - mean*r
                nc.vector.tensor_scalar(b1[:], mean[:], r[:], -1.0,
                                        Alu.mult, Alu.mult)
                nc.vector.tensor_scalar(b1[:], b1[:], 7.0, None, Alu.add)

                am = minis.tile([P, NCH2], f32)
                for h in range(NCH2):
                    sl = slice(h * CH2, (h + 1) * CH2)
                    z = zp.tile([P, CH2], f16, tag="z")
                    # z = w16*r + b1   (ts dual, f16, 4x)
                    nc.vector.tensor_scalar(z[:], w16[:, sl], r[:], b1[:],
                                            Alu.mult, Alu.add)
                    # fused: |w - s*clip(round(z),0,14)-7 .. | with accum
                    nc.vector._custom_dve(vq, out=z[:], in0=z[:],
                                          in1=w16[:, sl],
                                          s0=RND, s1=sblk[:], imm2=RND7,
                                          accum_out=am[:, h:h + 1])

                # out[:, b] = sum of the half-accums
                nc.vector.tensor_reduce(out_sb[:, b:b + 1], am[:],
                                        mybir.AxisListType.X, Alu.add)

            nc.sync.dma_start(out_ext[:], out_sb[:])

    nc.compile()
    return nc


def _get_program():
    global _PROGRAM
    if _PROGRAM is None:
        _PROGRAM = _build()
    return _PROGRAM


def kernel(weight, scale):
    w = np.ascontiguousarray(np.asarray(weight, dtype=np.float32))
    s = np.ascontiguousarray(
        np.asarray(scale, dtype=np.float32)).reshape(CFULL, 1)
    assert w.shape == (CFULL, K), w.shape

    nc = _get_program()
    in_maps = [
        {"w": w[i * CSH:(i + 1) * CSH], "s": s[i * CSH:(i + 1) * CSH]}
        for i in range(NCORES)
    ]
    res = run_bass_kernel_spmd(nc, in_maps, list(range(NCORES)))
    total = 0.0
    for i in range(NCORES):
        total += res.results[i]["out"].astype(np.float64).sum()
    return np.float32(total)
